# revision 2
# baseline (speedup 1.0000x reference)
"""LSTM kernel for Trainium2, data-parallel over batch across 8 NeuronCores.

Reference semantics (T=512, B=64, D=H=1024):
    x_proj = X @ Wx + b                      # [T, B, 4H]
    per step: gates = x_proj[t] + h @ Wh
              i,f,o,g = split(gates); c = sig(f)*c + sig(i)*tanh(g)
              h = sig(o)*tanh(c)
Outputs: (outputs [T,B,H], Hf [B,H], Cf [B,H])

Sharding: batch 64 -> 8 cores x 8 samples. Weights replicated. Recurrence
fully local per core (no collectives).
"""

import sys

sys.path.insert(0, "/opt/trn_rl_repo")

import numpy as np

import concourse.bass as bass
import concourse.mybir as mybir
import concourse.tile as tile
from concourse import bacc
from concourse import bass_utils
from concourse.bass import ds, ts
from concourse.masks import make_identity

FP = mybir.dt.float32
AF = mybir.ActivationFunctionType

N_CORES = 8
H = 1024
D = 1024
G = 4 * H          # gate width 4096
NCH = G // 512     # 8 chunks of 512
KT = H // 128      # 8 k-tiles

_CACHE = {}


def build(T, B_c, has_bias, unroll=4):
    """Build the Bass module for one core (SPMD across 8)."""
    nc = bacc.Bacc("TRN2", target_bir_lowering=False, debug=False,
                   enable_asserts=False, num_devices=1)

    M = T * B_c                       # rows of x_proj per core
    MT = M // 128                     # m-tiles for phase 1

    # ---- DRAM I/O ----
    xt_d = nc.dram_tensor("xt", [D, M], FP, kind="ExternalInput").ap()      # X^T per core
    wx_d = nc.dram_tensor("wx", [D, G], FP, kind="ExternalInput").ap()
    wh_d = nc.dram_tensor("wh", [H, G], FP, kind="ExternalInput").ap()
    h0_d = nc.dram_tensor("h0", [B_c, H], FP, kind="ExternalInput").ap()
    c0_d = nc.dram_tensor("c0", [B_c, H], FP, kind="ExternalInput").ap()
    if has_bias:
        b_d = nc.dram_tensor("b", [1, G], FP, kind="ExternalInput").ap()
    out_d = nc.dram_tensor("out", [M, H], FP, kind="ExternalOutput").ap()   # h per step
    cf_d = nc.dram_tensor("cf", [B_c, H], FP, kind="ExternalOutput").ap()
    xp_d = nc.dram_tensor("xp", [M, G], FP, kind="Internal").ap()           # x_proj scratch

    with tile.TileContext(nc) as tc:
        import contextlib
        ctx = contextlib.ExitStack()
        with ctx:
            const = ctx.enter_context(tc.tile_pool(name="const", bufs=1))
            wpool = ctx.enter_context(tc.tile_pool(name="wts", bufs=1))
            state = ctx.enter_context(tc.tile_pool(name="state", bufs=1))
            xtp = ctx.enter_context(tc.tile_pool(name="xtp", bufs=KT + 2))
            evp = ctx.enter_context(tc.tile_pool(name="evp", bufs=3))
            xpp = ctx.enter_context(tc.tile_pool(name="xpp", bufs=2))
            tmp = ctx.enter_context(tc.tile_pool(name="tmp", bufs=2))
            psum = ctx.enter_context(tc.tile_pool(name="psum", bufs=3, space="PSUM"))
            tpsum = ctx.enter_context(tc.tile_pool(name="tpsum", bufs=2, space="PSUM"))

            ident = const.tile([128, 128], FP)
            make_identity(nc, ident)

            # big weights buffer, used for Wx in phase 1 then Wh in phase 2
            wts = wpool.tile([128, KT, G], FP)          # [128, 8, 4096]

            if has_bias:
                bsb = const.tile([1, G], FP)
                nc.sync.dma_start(bsb, b_d)

            # ---------------- Phase 1: x_proj = X @ Wx (+ b) ----------------
            nc.sync.dma_start(wts, wx_d.rearrange("(k p) g -> p k g", p=128))
            for m in range(MT):
                xts = []
                for k in range(KT):
                    xt_t = xtp.tile([128, 128], FP, tag="xt")
                    nc.sync.dma_start(xt_t, xt_d[ts(k, 128), ts(m, 128)])
                    xts.append(xt_t)
                for n in range(NCH):
                    ps = psum.tile([128, 512], FP, tag="ps1")
                    for k in range(KT):
                        nc.tensor.matmul(ps, xts[k], wts[:, k, ts(n, 512)],
                                         start=(k == 0), stop=(k == KT - 1))
                    ev = evp.tile([128, 512], FP, tag="ev")
                    if has_bias:
                        nc.vector.tensor_add(ev, ps,
                                             bsb[0:1, ts(n, 512)].to_broadcast((128, 512)))
                    else:
                        nc.scalar.copy(ev, ps)
                    nc.sync.dma_start(xp_d[ts(m, 128), ts(n, 512)], ev)

            # ---------------- Phase 2: recurrence ----------------
            # load Wh over Wx (WAR handled by tile framework)
            nc.sync.dma_start(wts, wh_d.rearrange("(k p) g -> p k g", p=128))

            c_sb = state.tile([B_c, H], FP)         # cell state
            hst = state.tile([128, H], FP)          # h staging, rows B_c..127 stay zero
            hT = state.tile([128, KT * B_c], FP)    # h^T tiles: [:, k*B_c:(k+1)*B_c]
            tcb = state.tile([B_c, H], FP)          # tanh(c)
            gts = state.tile([B_c, G], FP)          # activated gates
            t1 = state.tile([B_c, H], FP)
            t2 = state.tile([B_c, H], FP)

            nc.gpsimd.memset(hst, 0.0)
            nc.sync.dma_start(c_sb, c0_d)
            nc.sync.dma_start(hst[0:B_c, :], h0_d)

            def transpose_h():
                # hT[:, k*B_c:(k+1)*B_c] = hst[:, k*128:(k+1)*128]^T (via E8 matmul)
                for k in range(KT):
                    pt = tpsum.tile([128, B_c], FP, tag="pt")
                    nc.tensor.matmul(pt, hst[:, ts(k, 128)], ident[:, 0:B_c],
                                     start=True, stop=True)
                    nc.vector.tensor_copy(hT[:, ts(k, B_c)], pt)

            transpose_h()

            def step(row0):
                """one LSTM step; row0 = dynamic row offset (t*B_c) into xp/out"""
                xp = xpp.tile([B_c, G], FP, tag="xp")
                nc.sync.dma_start(xp, xp_d[ds(row0, B_c), :])
                for n in range(NCH):
                    ps = psum.tile([128, 512], FP, tag="ps2")
                    # inject x_proj chunk: E8^T @ xp = xp  (K = B_c)
                    nc.tensor.matmul(ps[0:B_c, :], ident[0:B_c, 0:B_c],
                                     xp[:, ts(n, 512)], start=True, stop=False)
                    for k in range(KT):
                        nc.tensor.matmul(ps[0:B_c, :], hT[:, ts(k, B_c)],
                                         wts[:, k, ts(n, 512)],
                                         start=False, stop=(k == KT - 1))
                    func = AF.Sigmoid if n < 6 else AF.Tanh
                    nc.scalar.activation(gts[:, ts(n, 512)], ps[0:B_c, :], func)
                # c = sig(f)*c + sig(i)*tanh(g) ; h = sig(o)*tanh(c)
                nc.vector.tensor_mul(t1, gts[:, 0:H], gts[:, 3 * H:4 * H])   # i*g
                nc.vector.tensor_mul(t2, gts[:, H:2 * H], c_sb)              # f*c
                nc.vector.tensor_add(c_sb, t1, t2)
                nc.scalar.activation(tcb, c_sb, AF.Tanh)
                nc.vector.tensor_mul(hst[0:B_c, :], gts[:, 2 * H:3 * H], tcb)
                nc.sync.dma_start(out_d[ds(row0, B_c), :], hst[0:B_c, :])
                transpose_h()

            if T % unroll == 0 and T > unroll:
                with tc.For_i(0, T, unroll) as it:
                    for u in range(unroll):
                        step((it + u) * B_c)
            else:
                for t in range(T):
                    step(t * B_c)

            nc.sync.dma_start(cf_d, c_sb)

    nc.compile()
    return nc


def kernel(**inputs):
    inputs = {k: np.asarray(v) for k, v in inputs.items()}
    X = inputs["inputs"].astype(np.float32)           # [T, B, D]
    T, B, _D = X.shape
    B_c = B // N_CORES
    Wx = np.concatenate([inputs["W_xi"], inputs["W_xf"],
                         inputs["W_xo"], inputs["W_xc"]], axis=1).astype(np.float32)
    Wh = np.concatenate([inputs["W_hi"], inputs["W_hf"],
                         inputs["W_ho"], inputs["W_hc"]], axis=1).astype(np.float32)
    b = np.concatenate([inputs["b_i"], inputs["b_f"],
                        inputs["b_o"], inputs["b_c"]]).astype(np.float32)
    has_bias = bool(np.any(b))

    key = (T, B_c, has_bias)
    if key not in _CACHE:
        _CACHE[key] = build(T, B_c, has_bias)
    nc = _CACHE[key]

    in_maps = []
    for c in range(N_CORES):
        sl = slice(c * B_c, (c + 1) * B_c)
        Xc = X[:, sl, :].reshape(T * B_c, D)
        m = {
            "xt": np.ascontiguousarray(Xc.T),
            "wx": Wx, "wh": Wh,
            "h0": np.ascontiguousarray(inputs["H0"][sl]).astype(np.float32),
            "c0": np.ascontiguousarray(inputs["C0"][sl]).astype(np.float32),
        }
        if has_bias:
            m["b"] = b.reshape(1, -1)
        in_maps.append(m)

    res = bass_utils.run_bass_kernel_spmd(nc, in_maps, core_ids=list(range(N_CORES)))

    outputs = np.empty((T, B, H), np.float32)
    Cf = np.empty((B, H), np.float32)
    for c in range(N_CORES):
        sl = slice(c * B_c, (c + 1) * B_c)
        outputs[:, sl, :] = res.results[c]["out"].reshape(T, B_c, H)
        Cf[sl] = res.results[c]["cf"]
    return outputs, outputs[-1].copy(), Cf
